# revision 23
# baseline (speedup 1.0000x reference)
"""Trainium2 Bass kernel for nn_MemoryCell (causal linear attention memory cell).

Math: the reference's sequential scan
    mem += outer(k_t, v_t); zeta += k_t; y_t = (q_t @ mem) / (q_t . zeta)
is exactly causal linear attention:
    y_t = sum_{s<=t} (q_t.k_s) v_s / sum_{s<=t} (q_t.k_s)
computed here as chunked attention over superchunks of SC=512 steps:
    y = Q @ Mhat + tril(Q K^T) @ Vhat      (Vhat has an all-ones column
                                            which produces the normalizer)
    Mhat += K^T Vhat  per superchunk.

Sharding (8 cores, feature/tensor parallel per the hint):
  each core computes its 256-wide slice of gated Q^T/K^T (projection from
  full x with its W row-slice), all-gathers them (bf16 wire), and computes
  its 256-wide V/Mhat/y column slice locally. y slices are concatenated on
  the host.

dtypes: x/W/q/k wire in bf16 (the sigmoid gates are 0.5 +- 6e-4, and the
output is insensitive to their low bits; measured end-to-end rel err vs a
float64 oracle stays ~1e-3, far inside the 2e-2 family gate). V/Mhat/AT and
all PSUM accumulation stay fp32 (matmuls in fp32r = full-rate reduced
precision).
"""

import os

import numpy as np

T, D = 4096, 2048
NCORE = 8
DV = D // NCORE          # 256: v-columns per core
DVE = DV + 2             # v-columns + ones column + pad (fp32r needs even N)
P = 128
KD = D // P              # 16 feature tiles
SC = 512                 # superchunk length (t)
NSC = T // SC            # 8
NBLK = SC // P           # 4 blocks per superchunk
TCH = 512                # projection t-chunk
NTCH = T // TCH

_CACHE = {}


def _cs_factors():
    idx = np.arange(D // 2, dtype=np.float32)
    thetas = np.float32(10000.0) ** (np.float32(-2.0) * idx)
    pos = np.arange(T, dtype=np.float32)
    ang = pos[:, None] * thetas[None, :]
    cos = np.repeat(np.cos(ang), 2, axis=-1)
    sin = np.repeat(np.sin(ang), 2, axis=-1)
    return (cos + sin).astype(np.float32)


def _build_nc():
    import concourse.bacc as bacc
    import concourse.mybir as mybir
    import concourse.tile as tile
    from concourse.bass import ts
    from concourse.masks import make_upper_triangular

    f32 = mybir.dt.float32
    f32r = mybir.dt.float32r
    bf16 = mybir.dt.bfloat16
    SIG = mybir.ActivationFunctionType.Sigmoid
    MUL = mybir.AluOpType.mult

    def r(ap):
        return ap.bitcast(f32r)

    nc = bacc.Bacc(num_devices=NCORE)

    xT = nc.dram_tensor("xT", [D, T], bf16, kind="ExternalInput")
    wqT = nc.dram_tensor("wqT", [D, DV], bf16, kind="ExternalInput")
    wkT = nc.dram_tensor("wkT", [D, DV], bf16, kind="ExternalInput")
    wvT = nc.dram_tensor("wvT", [D, DV], bf16, kind="ExternalInput")
    csT = nc.dram_tensor("csT", [DV, T], f32, kind="ExternalInput")
    y_out = nc.dram_tensor("y", [T, DV], f32, kind="ExternalOutput")

    xTv = xT[:, :].rearrange("(k p) t -> p k t", p=P)     # [128, 16, T]
    wqv = wqT[:, :].rearrange("(k p) n -> p k n", p=P)    # [128, 16, 256]
    wkv = wkT[:, :].rearrange("(k p) n -> p k n", p=P)
    wvv = wvT[:, :].rearrange("(k p) n -> p k n", p=P)
    csv = csT[:, :].rearrange("(k p) t -> p k t", p=P)    # [128, 2, T]

    with tile.TileContext(nc) as tc:
        with (
            tc.tile_pool(name="const", bufs=1) as constp,
            tc.tile_pool(name="dram", bufs=1, space="DRAM") as dramp,
            tc.tile_pool(name="mhat", bufs=1) as mhatp,
        ):
            triu = constp.tile([P, P], f32)
            make_upper_triangular(nc, triu[:], val=1.0, diag=True)

            TH = T // 2
            q_bounce = [dramp.tile([DV, TH], bf16, name=f"q_bounce{h}") for h in range(2)]
            k_bounce = [dramp.tile([DV, TH], bf16, name=f"k_bounce{h}") for h in range(2)]
            q_gath = [
                dramp.tile([D, TH], bf16, addr_space="Shared", name=f"q_gath{h}")
                for h in range(2)
            ]
            k_gath = [
                dramp.tile([D, TH], bf16, addr_space="Shared", name=f"k_gath{h}")
                for h in range(2)
            ]

            mhat = mhatp.tile([P, KD, DVE], bf16)         # [128, 16, 258]
            nc.vector.memset(mhat[:], 0.0)
            wv_sb = constp.tile([P, KD, DV], bf16)
            nc.sync.dma_start(wv_sb[:], wvv)

            # ---------------- Phase 1: q/k projections + gating ----------------
            with (
                tc.tile_pool(name="w", bufs=1) as wp,
                tc.tile_pool(name="xin", bufs=2) as xp,
                tc.tile_pool(name="csp", bufs=2) as csp,
                tc.tile_pool(name="qk", bufs=3) as qkp,
                tc.tile_pool(name="pj_ps", bufs=2, space="PSUM") as pjps,
            ):
                wq_sb = wp.tile([P, KD, DV], bf16)
                nc.sync.dma_start(wq_sb[:], wqv)
                wk_sb = wp.tile([P, KD, DV], bf16)
                nc.sync.dma_start(wk_sb[:], wkv)

                def all_gather(src, dst):
                    nc.gpsimd.collective_compute(
                        "AllGather",
                        mybir.AluOpType.bypass,
                        replica_groups=[list(range(NCORE))],
                        ins=[src.opt()],
                        outs=[dst.opt()],
                    )

                NCH = NTCH // 2  # t-chunks per half
                for c in range(NTCH):
                    h, ch = divmod(c, NCH)
                    xt = xp.tile([P, KD, TCH], bf16, tag="xt")
                    nc.sync.dma_start(xt[:], xTv[:, :, ts(c, TCH)])
                    cst = csp.tile([P, 2, TCH], f32, tag="cst")
                    nc.sync.dma_start(cst[:], csv[:, :, ts(c, TCH)])

                    for w_sb, bounce in ((wq_sb, q_bounce[h]), (wk_sb, k_bounce[h])):
                        g = qkp.tile([P, 2, TCH], bf16, tag="g")
                        for do in range(2):
                            ps = pjps.tile([P, TCH], f32, tag="pj")
                            for k in range(KD):
                                nc.tensor.matmul(
                                    ps[:],
                                    w_sb[:, k, ts(do, P)],
                                    xt[:, k, :],
                                    start=(k == 0),
                                    stop=(k == KD - 1),
                                )
                            nc.vector.tensor_mul(g[:, do, :], ps[:], cst[:, do, :])
                            nc.scalar.activation(g[:, do, :], g[:, do, :], SIG, scale=1.0 / D)
                        nc.sync.dma_start(
                            bounce[:, :].rearrange("(k p) t -> p k t", p=P)[:, :, ts(ch, TCH)],
                            g[:],
                        )

                    if c == NCH - 1:  # first halves done -> gathers overlap rest
                        all_gather(q_bounce[0], q_gath[0])
                        all_gather(k_bounce[0], k_gath[0])

                all_gather(q_bounce[1], q_gath[1])
                all_gather(k_bounce[1], k_gath[1])

            qgv = [g[:, :].rearrange("(k p) t -> p k t", p=P) for g in q_gath]
            kgv = [g[:, :].rearrange("(k p) t -> p k t", p=P) for g in k_gath]

            # ---------------- Phase 3: V projection + chunked causal linear attention ----------------
            with (
                tc.tile_pool(name="asc", bufs=3) as ap_,
                tc.tile_pool(name="bblk", bufs=5) as bp_,
                tc.tile_pool(name="kn", bufs=6) as knp,
                tc.tile_pool(name="xin2", bufs=3) as xp2,
                tc.tile_pool(name="vhat", bufs=3) as vhp,
                tc.tile_pool(name="atsb", bufs=5) as atp,
                tc.tile_pool(name="ysb", bufs=3) as yp_,
                tc.tile_pool(name="rec", bufs=3) as recp,
                tc.tile_pool(name="pv_ps", bufs=2, space="PSUM") as pvps,
                tc.tile_pool(name="at_ps", bufs=2, space="PSUM") as atps,
                tc.tile_pool(name="y_ps", bufs=2, space="PSUM") as yps,
                tc.tile_pool(name="d_ps", bufs=2, space="PSUM") as dps,
            ):
                for s in range(NSC):
                    h, sh = divmod(s, NSC // 2)  # half index, superchunk-in-half
                    # V projection for this superchunk (independent of gathers)
                    xt_s = xp2.tile([P, KD, SC], bf16, tag="xt2")
                    nc.sync.dma_start(xt_s[:], xTv[:, :, ts(s, SC)])
                    vhat_s = vhp.tile([P, NBLK, DVE], bf16, tag="vh")
                    nc.vector.memset(vhat_s[:, :, DV:DVE], 1.0)
                    for tt in range(NBLK):
                        psv = pvps.tile([P, DV], f32, tag="pv")
                        for k in range(KD):
                            nc.tensor.matmul(
                                psv[:],
                                xt_s[:, k, ts(tt, P)],
                                wv_sb[:, k, :],
                                start=(k == 0),
                                stop=(k == KD - 1),
                            )
                        nc.vector.tensor_copy(vhat_s[:, tt, 0:DV], psv[:])

                    # a (gated q^T) for the whole superchunk: [128, 16, 512]
                    a_sc = ap_.tile([P, KD, SC], bf16, tag="a")
                    nc.sync.dma_start(a_sc[:], qgv[h][:, :, ts(sh, SC)])

                    # b (gated k^T) per block; kn = K in normal [t, d] layout
                    # via DMA-transpose (scalar-engine HWDGE ring, away from
                    # the bulk copies to avoid xbar-mode thrash)
                    b_blks, kn_blks = [], []
                    for i in range(NBLK):
                        blk_h = sh * NBLK + i
                        b_i = bp_.tile([P, KD, P], bf16, tag="b")
                        nc.sync.dma_start(b_i[:], kgv[h][:, :, ts(blk_h, P)])
                        kn_i = knp.tile([P, KD * P], bf16, tag="kn")
                        nc.scalar.dma_start(
                            kn_i[:], k_gath[h][:, ts(blk_h, P)], transpose=True
                        )
                        b_blks.append(b_i)
                        kn_blks.append(kn_i)

                    # AT rows: AT[j] = (K_j Q^T)[t' x t-span], masked to t >= t'
                    at_rows = []
                    for j in range(NBLK):
                        span = (NBLK - j) * P
                        ps_at = atps.tile([P, NBLK * P], f32, tag="at_ps")
                        for k in range(KD):
                            nc.tensor.matmul(
                                ps_at[:, 0:span],
                                b_blks[j][:, k, :],
                                a_sc[:, k, j * P : NBLK * P],
                                start=(k == 0),
                                stop=(k == KD - 1),
                            )
                        at_j = atp.tile([P, NBLK * P], bf16, tag="at")
                        nc.vector.tensor_mul(at_j[:, 0:P], ps_at[:, 0:P], triu[:])
                        if span > P:
                            nc.vector.tensor_copy(at_j[:, P:span], ps_at[:, P:span])
                        at_rows.append(at_j)

                    # y blocks: inter (Q Mhat) + intra (AT^T Vhat); divide by ones-col
                    for i in range(NBLK):
                        blk = s * NBLK + i
                        ps_y = yps.tile([P, DVE], f32, tag="y")
                        mms = []
                        if s > 0:
                            for k in range(KD):
                                mms.append((a_sc[:, k, ts(i, P)], mhat[:, k, :]))
                        for j in range(i + 1):
                            mms.append(
                                (at_rows[j][:, ts(i - j, P)], vhat_s[:, j, :])
                            )
                        for mi, (l_, r_) in enumerate(mms):
                            nc.tensor.matmul(
                                ps_y[:], l_, r_, start=(mi == 0), stop=(mi == len(mms) - 1)
                            )
                        rec = recp.tile([P, 1], f32, tag="rec")
                        nc.vector.reciprocal(rec[:], ps_y[:, DV : DV + 1])
                        y_sb = yp_.tile([P, DV], f32, tag="ysb")
                        nc.vector.tensor_scalar(y_sb[:], ps_y[:, 0:DV], rec[:], None, MUL)
                        nc.sync.dma_start(y_out[ts(blk, P), :], y_sb[:])

                    # Mhat += K^T Vhat for this superchunk
                    for k in range(KD):
                        ps_d = dps.tile([P, DVE], f32, tag="d")
                        for i in range(NBLK):
                            nc.tensor.matmul(
                                ps_d[:],
                                kn_blks[i][:, ts(k, P)],
                                vhat_s[:, i, :],
                                start=(i == 0),
                                stop=(i == NBLK - 1),
                            )
                        nc.vector.tensor_add(mhat[:, k, :], mhat[:, k, :], ps_d[:])

    nc.compile()
    return nc


def kernel(x, Wq, Wk, Wv):
    import ml_dtypes

    from concourse.bass_utils import run_bass_kernel_spmd

    x = np.ascontiguousarray(np.asarray(x, dtype=np.float32))
    Wq = np.asarray(Wq, dtype=np.float32)
    Wk = np.asarray(Wk, dtype=np.float32)
    Wv = np.asarray(Wv, dtype=np.float32)

    bf = ml_dtypes.bfloat16
    csT = np.ascontiguousarray(_cs_factors().T)           # [D, T]
    xT = np.ascontiguousarray(x.T).astype(bf)             # [D, T]

    in_maps = []
    for m in range(NCORE):
        sl = slice(m * DV, (m + 1) * DV)
        in_maps.append(
            {
                "xT": xT,
                "wqT": np.ascontiguousarray(Wq[sl, :].T).astype(bf),
                "wkT": np.ascontiguousarray(Wk[sl, :].T).astype(bf),
                "wvT": np.ascontiguousarray(Wv[sl, :].T).astype(bf),
                "csT": np.ascontiguousarray(csT[sl, :]),
            }
        )

    if "nc" not in _CACHE:
        _CACHE["nc"] = _build_nc()
    nc = _CACHE["nc"]

    trace = bool(int(os.environ.get("KERNEL_TRACE", "0")))
    res = run_bass_kernel_spmd(nc, in_maps, core_ids=list(range(NCORE)), trace=trace)
    _CACHE["last_result"] = res

    return np.concatenate([res.results[m]["y"] for m in range(NCORE)], axis=1)
